# revision 1
# baseline (speedup 1.0000x reference)
"""Trainium2 Bass kernel for NGCF-style embedding propagation (8 NeuronCores).

Math (reference, with A = adj / (sqrt(row_sum*col_sum)+eps)):
  updated_user = LReLU(A.T @ (item@W1) + (item * (A.T @ user)) @ W2 + user)
  updated_item = LReLU(A   @ (user@W1) + (user * (A   @ item)) @ W2 + item)

Row-shard adj across 8 cores (1024 rows each). Per core, with
Xr = s_r*[item@W1, user] (own rows) and Xc = s_c*[user@W1, item] (all cols):
  P_pre = adj.T @ Xr  -> ReduceScatter over user blocks
  Q     = adj @ Xc    -> local (own rows)

Schedule (the point of this rewrite): loads are PANEL-major (8 panels of
1024 columns x 8 row-blocks). Per chunk the fp32 data is cast to a resident
fp16 natural cache (ACT, fused row-sum accumulation), PE-transposed into a
double-buffered per-panel adjT buffer (staged PSUM->SBUF on DVE), and column
partial sums accumulate on PE. After each panel its 32KiB column-sum slice is
AllReduced; s_c and Xc for that panel are built while the next panel loads,
and Q runs lagged one panel as natural [row, d] accumulation in PSUM. So the
transpose, colsums, Q, and e1=user@W1 all hide inside the 93us adjacency
load. Phase B is only: s_r, Xr, P (natural cache resident), ReduceScatter,
and the two finish loops.
"""

import numpy as np

N = 8192
D = 64
NCORES = 8
U = N // NCORES          # rows per core = 1024
UB = U // 128            # 128-row blocks per core = 8
CB = N // 128            # 128-col blocks = 64
PAN = 8                  # column panels
PCB = CB // PAN          # col blocks per panel = 8
PW = PCB * 128           # panel width = 1024

_CACHE = {}


def _build(dbg=False, single=False):
    import concourse.bass as bass
    import concourse.bacc as bacc
    import concourse.mybir as mybir
    import concourse.tile as tile
    from concourse import masks

    f32 = mybir.dt.float32
    f16 = mybir.dt.float16
    bf16 = mybir.dt.bfloat16
    AF = mybir.ActivationFunctionType
    ALU = mybir.AluOpType
    ds = bass.ds

    nc = bacc.Bacc("TRN2", target_bir_lowering=False, debug=False,
                   num_devices=(1 if single else NCORES), enable_asserts=False)

    adj = nc.dram_tensor("adj", [U, N], f32, kind="ExternalInput").ap()
    user_full = nc.dram_tensor("user_full", [N, D], f32, kind="ExternalInput").ap()
    item_full = nc.dram_tensor("item_full", [N, D], f32, kind="ExternalInput").ap()
    user_own = nc.dram_tensor("user_own", [U, D], f32, kind="ExternalInput").ap()
    item_own = nc.dram_tensor("item_own", [U, D], f32, kind="ExternalInput").ap()
    w1 = nc.dram_tensor("w1", [D, D], f32, kind="ExternalInput").ap()
    w2 = nc.dram_tensor("w2", [D, D], f32, kind="ExternalInput").ap()
    upd_user = nc.dram_tensor("upd_user", [U, D], f32, kind="ExternalOutput").ap()
    upd_item = nc.dram_tensor("upd_item", [U, D], f32, kind="ExternalOutput").ap()

    groups = [list(range(NCORES))]

    with tile.TileContext(nc) as tc:
        with (
            tc.tile_pool(name="persist", bufs=1) as persist,
            tc.tile_pool(name="ld", bufs=3) as ldp,
            tc.tile_pool(name="embld", bufs=1) as embld,
            tc.tile_pool(name="small", bufs=2) as small,
            tc.tile_pool(name="pstp", bufs=2) as pstp,
            tc.tile_pool(name="psum_small", bufs=2, space="PSUM") as psum_small,
            tc.tile_pool(name="ps2k", bufs=2, space="PSUM") as ps2k,
            tc.tile_pool(name="psum_big", bufs=1, space="PSUM") as psum_big,
            tc.tile_pool(name="dram", bufs=1, space="DRAM") as dram,
        ):
            # ---------------- persistent SBUF tiles
            cache = persist.tile([128, UB, N], f16)          # 128 KiB/part
            adjt = persist.tile([128, 2, PCB, UB, 128], f16)  # 32 KiB
            ei = persist.tile([128, CB, 2 * D], f16)         # 16 KiB (e1|item)
            xc = persist.tile([128, 2, PCB, 2 * D], f16)     # 4 KiB
            uown = persist.tile([128, UB, D], f16)           # 1
            iown = persist.tile([128, UB, D], f16)           # 1
            xr = persist.tile([128, UB, 2 * D], f16)         # 2 (x0r then Xr)
            rowsum_parts = persist.tile([128, UB, PAN], f32)
            s_r = persist.tile([128, UB], f32)
            s_c = persist.tile([128, CB], f32)
            out_stage = persist.tile([128, UB, D], f32)      # 2
            w1_hf = persist.tile([D, D], f16)
            w2_hf = persist.tile([D, D], f16)
            ones_hf = persist.tile([128, 1], f16)
            onerow = persist.tile([1, 128], f16)
            zrow = persist.tile([1, 128], f16)
            ident = persist.tile([128, 128], f16)
            identbf = persist.tile([128, 128], bf16)

            psum_q = psum_big.tile([128, UB, 128], f32)      # 2 banks
            psum_col = psum_big.tile([128, CB], f32)         # 1 bank

            nc.gpsimd.memset(ones_hf[:], 1.0)
            nc.gpsimd.memset(onerow[:], 1.0)
            nc.gpsimd.memset(zrow[:], 0.0)
            masks.make_identity(nc, ident[:])
            masks.make_identity(nc, identbf[:])

            # W1/W2 -> fp16
            wld = embld.tile([D, 2 * D], f32, tag="ue")
            nc.gpsimd.dma_start(wld[:, 0:D], w1)
            nc.gpsimd.dma_start(wld[:, D:2 * D], w2)
            nc.vector.tensor_copy(w1_hf[:], wld[:, 0:D])
            nc.vector.tensor_copy(w2_hf[:], wld[:, D:2 * D])

            # own embeddings (fp32, one DMA each)
            uo_view = user_own.rearrange("(ub p) d -> p ub d", p=128)
            io_view = item_own.rearrange("(ub p) d -> p ub d", p=128)
            nc.gpsimd.dma_start(uown[:], uo_view)
            nc.gpsimd.dma_start(iown[:], io_view)

            # prime accumulator banks (start=True zero-matmul sets
            # has_written across each region; later matmuls accumulate-only).
            nc.tensor.matmul(psum_col[:], onerow[:], zrow[:, 0:CB],
                             start=True, stop=False, skip_group_check=True)
            for qb in range(UB):
                nc.tensor.matmul(psum_q[:, qb], onerow[:], zrow[:],
                                 start=True, stop=False,
                                 skip_group_check=True)

            # x0r = [iown@W1, uown] (unscaled); s_r applied in phase B
            for ub in range(UB):
                ib = small.tile([128, D], f16, tag="ib")
                nc.vector.tensor_copy(ib[:], iown[:, ub])
                pt = psum_small.tile([D, 128], f16, tag="pe")
                nc.tensor.transpose(pt[:], ib[:], ident[:])
                ibt = small.tile([D, 128], f16, tag="ibt")
                nc.vector.tensor_copy(ibt[:], pt[:])
                pe = psum_small.tile([128, D], f32, tag="pe")
                nc.tensor.matmul(pe[:], ibt[:], w1_hf[:], start=True, stop=True)
                nc.scalar.activation(xr[:, ub, 0:D], pe[:], AF.Copy)
                nc.vector.tensor_copy(xr[:, ub, D:2 * D], uown[:, ub])

            # ei = [user@W1 | item] in natural c-order (overlaps phase A).
            # user side: per 128-block transpose -> matmul; batched in psum.
            uf_view = user_full.rearrange("(b c p) d -> p b c d", p=128, c=4)
            if_view = item_full.rearrange("(b c p) d -> p b c d", p=128, c=4)
            for b in range(CB // 4):
                ue = embld.tile([128, 4, D], f32, tag="ue")
                nc.gpsimd.dma_start(ue[:], uf_view[:, b])
                uhf = small.tile([128, 4, D], f16, tag="uhf")
                nc.vector.tensor_copy(uhf[:], ue[:])
                ptu = psum_small.tile([D, 4, 128], f16, tag="pe")
                for i in range(4):
                    nc.tensor.transpose(ptu[:, i], uhf[:, i], ident[:])
                ubt = small.tile([D, 4, 128], f16, tag="ubt")
                nc.vector.tensor_copy(ubt[:], ptu[:])
                peu = psum_small.tile([128, 4, D], f32, tag="pe")
                for i in range(4):
                    nc.tensor.matmul(peu[:, i], ubt[:, i], w1_hf[:],
                                     start=True, stop=True)
                nc.scalar.activation(ei[:, b * 4:(b + 1) * 4, 0:D], peu[:],
                                     AF.Copy)
            for b in range(CB // 4):
                ie = embld.tile([128, 4, D], f32, tag="ue")
                nc.gpsimd.dma_start(ie[:], if_view[:, b])
                nc.vector.tensor_copy(ei[:, b * 4:(b + 1) * 4, D:2 * D], ie[:])

            # per-panel column-sum AllReduce buffers
            col_in = []
            col_out = []
            for _pn in range(PAN):
                ci = dram.tile([128, PCB], f32, name=f"col_in{_pn}")
                co = dram.tile([128, PCB], f32, addr_space="Shared",
                               name=f"col_out{_pn}")
                col_in.append(ci)
                col_out.append(co)

            adj_v = adj.rearrange("(ub p) n -> p ub n", p=128)

            def emit_q(panel):
                """Q matmuls for a completed panel (lagged): natural [r, d]
                accumulation, one matmul per (cb, ub)."""
                buf = panel % 2
                for j in range(PCB):
                    cb = panel * PCB + j
                    for ub in range(UB):
                        nc.tensor.matmul(
                            psum_q[:, ub], adjt[:, buf, j, ub],
                            xc[:, buf, j],
                            start=False, stop=(cb == CB - 1 and ub == UB - 1),
                            skip_group_check=True)

            # ---------------- phase A: panel-major streaming
            for panel in range(PAN):
                cs = slice(panel * PW, (panel + 1) * PW)
                for ub in range(UB):
                    ld = ldp.tile([128, PW], f32, tag="ld")
                    nc.sync.dma_start(ld[:], adj_v[:, ub, cs])
                    # cast -> resident fp16 cache, fused row-sum accumulation
                    nc.scalar.activation(
                        cache[:, ub, cs], ld[:], AF.Copy,
                        accum_out=rowsum_parts[:, ub, panel:panel + 1])
                    # PE transposes -> PSUM (fp16), staged to adjT on DVE
                    pst = ps2k.tile([128, PCB, 128], f16, tag="s2k")
                    for j in range(PCB):
                        c0 = panel * PW + j * 128
                        nc.tensor.transpose(pst[:, j],
                                            cache[:, ub, c0:c0 + 128],
                                            ident[:])
                    nc.vector.tensor_copy(adjt[:, panel % 2, :, ub], pst[:])
                    # column partial sums (accumulate over ub)
                    for j in range(PCB):
                        cb = panel * PCB + j
                        c0 = cb * 128
                        nc.tensor.matmul(
                            psum_col[:, cb:cb + 1],
                            cache[:, ub, c0:c0 + 128], ones_hf[:],
                            start=False, stop=(panel == PAN - 1 and ub == UB - 1),
                            skip_group_check=True)
                    # lagged Q for previous panel: burst once AR is done
                    if panel > 0 and ub == 5:
                        emit_q(panel - 1)

                # panel column sums complete -> AllReduce -> s_c -> Xc
                csl = slice(panel * PCB, (panel + 1) * PCB)
                col_sb = small.tile([128, PCB], f32, tag="colsb")
                nc.vector.tensor_copy(col_sb[:], psum_col[:, csl])
                nc.gpsimd.dma_start(col_in[panel][:], col_sb[:])
                if single:
                    nc.gpsimd.dma_start(col_out[panel][:], col_in[panel][:])
                else:
                    nc.gpsimd.collective_compute(
                        "AllReduce", mybir.AluOpType.add, replica_groups=groups,
                        ins=[col_in[panel].opt()], outs=[col_out[panel].opt()])
                colsb2 = small.tile([128, PCB], f32, tag="cs2")
                nc.gpsimd.dma_start(colsb2[:], col_out[panel][:])
                sqc = small.tile([128, PCB], f32, tag="sqc")
                nc.scalar.sqrt(sqc[:], colsb2[:])
                nc.vector.reciprocal(s_c[:, csl], sqc[:])
                for j in range(PCB):
                    cb = panel * PCB + j
                    nc.vector.tensor_scalar(
                        xc[:, panel % 2, j], ei[:, cb], s_c[:, cb:cb + 1],
                        None, ALU.mult)

            # drain Q for the final panel
            emit_q(PAN - 1)

            # ---------------- phase B
            # s_r and Xr
            rowsum = small.tile([128, UB], f32, tag="sqr")
            for ub in range(UB):
                nc.vector.tensor_reduce(rowsum[:, ub:ub + 1],
                                        rowsum_parts[:, ub],
                                        mybir.AxisListType.X, ALU.add)
            sqr = small.tile([128, UB], f32, tag="sqr2")
            nc.scalar.sqrt(sqr[:], rowsum[:])
            nc.vector.reciprocal(s_r[:], sqr[:])
            for ub in range(UB):
                nc.scalar.activation(xr[:, ub], xr[:, ub], AF.Copy,
                                     scale=s_r[:, ub:ub + 1])

            # P: stationary Xr[ub], moving natural cache; out P^T blocks.
            p_in = dram.tile([CB, 128, 128], bf16)
            p_out = dram.tile([UB, 128, 128], bf16)
            uu_view = upd_user.rearrange("(ub p) d -> p ub d", p=128)
            ui_view = upd_item.rearrange("(ub p) d -> p ub d", p=128)

            def emit_p_sub(sub, eng):
                pp = ps2k.tile([128, 512], f32, tag="s2k")
                for ub in range(UB):
                    nc.tensor.matmul(
                        pp[:], xr[:, ub], cache[:, ub, sub * 512:(sub + 1) * 512],
                        start=(ub == 0), stop=(ub == UB - 1),
                        skip_group_check=True)
                pst = pstp.tile([128, 512], bf16, tag="pst")
                if eng is None:
                    nc.scalar.activation(pst[:], pp[:], AF.Copy)
                else:
                    eng.tensor_copy(pst[:], pp[:])
                blk = slice(sub * 4, (sub + 1) * 4)
                nc.sync.dma_start(p_in[blk].rearrange("b d c -> d b c"), pst[:])

            def emit_item_finish(ub):
                """out_item[ub] = LReLU(s_r*(q0 + (q1*uown)@W2) + iown)"""
                g = small.tile([128, D], f16, tag="g")
                nc.vector.tensor_mul(g[:], psum_q[:, ub, D:2 * D], uown[:, ub])
                ptg = psum_small.tile([D, 128], f16, tag="pe")
                nc.tensor.transpose(ptg[:], g[:], ident[:])
                gt = small.tile([D, 128], f16, tag="ibt")
                nc.scalar.activation(gt[:], ptg[:], AF.Copy)
                pg = psum_small.tile([128, D], f32, tag="pe")
                nc.tensor.matmul(pg[:], gt[:], w2_hf[:], start=True, stop=True)
                ta = small.tile([128, D], f32, tag="ft")
                nc.scalar.activation(ta[:], pg[:], AF.Copy)
                tb = small.tile([128, D], f32, tag="ft")
                nc.vector.scalar_tensor_tensor(
                    tb[:], psum_q[:, ub, 0:D], 1.0, ta[:], ALU.mult, ALU.add)
                tcm = small.tile([128, D], f32, tag="ft")
                nc.vector.scalar_tensor_tensor(
                    tcm[:], tb[:], s_r[:, ub:ub + 1], iown[:, ub],
                    ALU.mult, ALU.add)
                nc.vector.scalar_tensor_tensor(
                    out_stage[:, ub], tcm[:], 0.2, tcm[:], ALU.mult, ALU.max)

            for sub in range(16):
                emit_p_sub(sub, nc.vector if sub % 2 else None)
                if sub % 2 == 0:
                    emit_item_finish(sub // 2)

            nc.scalar.dma_start(ui_view[:], out_stage[:])
            if single:
                nc.sync.dma_start(p_out[:], p_in[0:UB])
            else:
                nc.gpsimd.collective_compute(
                    "ReduceScatter", mybir.AluOpType.add, replica_groups=groups,
                    ins=[p_in.opt()], outs=[p_out.opt()])

            # ---------------- finish user side (post-RS)
            pid = nc.vector.partition_id()
            rspv = p_out[:].rearrange("(h b) d c -> d h b c", h=2)
            for h in range(2):
                rh = pstp.tile([128, UB // 2, 128], bf16, tag="pst",
                               name=f"rsph{h}")
                nc.gpsimd.dma_start(rh[:], rspv[:, h])
                for k in range(UB // 2):
                    ub = h * (UB // 2) + k
                    rtt = psum_small.tile([128, 128], bf16, tag="pe")
                    nc.tensor.transpose(rtt[:], rh[:, k], identbf[:])
                    g2 = small.tile([128, D], f16, tag="g")
                    nc.vector.tensor_mul(g2[:], rtt[:, D:2 * D], iown[:, ub])
                    ptg2 = psum_small.tile([D, 128], f16, tag="pe")
                    nc.tensor.transpose(ptg2[:], g2[:], ident[:])
                    gt2 = small.tile([D, 128], f16, tag="ibt")
                    nc.scalar.activation(gt2[:], ptg2[:], AF.Copy)
                    pg2 = psum_small.tile([128, D], f32, tag="pe")
                    nc.tensor.matmul(pg2[:], gt2[:], w2_hf[:], start=True,
                                     stop=True)
                    if single:
                        sc_ap = s_c[:, ub:ub + 1]
                    else:
                        sc_ap = s_c[:, ds(pid * UB + ub, 1)]
                    t1 = small.tile([128, D], f32, tag="ft")
                    nc.vector.scalar_tensor_tensor(
                        t1[:], rtt[:, 0:D], sc_ap, uown[:, ub],
                        ALU.mult, ALU.add)
                    s2u = small.tile([128, D], f32, tag="ft")
                    nc.vector.scalar_tensor_tensor(
                        s2u[:], pg2[:], sc_ap, t1[:], ALU.mult, ALU.add)
                    nc.vector.scalar_tensor_tensor(
                        out_stage[:, ub], s2u[:], 0.2, s2u[:],
                        ALU.mult, ALU.max)
            nc.scalar.dma_start(uu_view[:], out_stage[:])

    nc.compile()
    return nc


def _get_nc(dbg=False):
    key = ("nc", dbg)
    if key not in _CACHE:
        _CACHE[key] = _build(dbg)
    return _CACHE[key]


def make_in_maps(user_embeddings, item_embeddings, adjacency_matrix, W1, W2):
    adj = np.ascontiguousarray(np.asarray(adjacency_matrix, dtype=np.float32))
    ue = np.ascontiguousarray(np.asarray(user_embeddings, dtype=np.float32))
    ie = np.ascontiguousarray(np.asarray(item_embeddings, dtype=np.float32))
    w1 = np.ascontiguousarray(np.asarray(W1, dtype=np.float32))
    w2 = np.ascontiguousarray(np.asarray(W2, dtype=np.float32))
    in_maps = []
    for k in range(NCORES):
        sl = slice(k * U, (k + 1) * U)
        in_maps.append({
            "adj": np.ascontiguousarray(adj[sl]),
            "user_full": ue,
            "item_full": ie,
            "user_own": np.ascontiguousarray(ue[sl]),
            "item_own": np.ascontiguousarray(ie[sl]),
            "w1": w1,
            "w2": w2,
        })
    return in_maps


def assemble(results):
    upd_user = np.concatenate([results[k]["upd_user"] for k in range(NCORES)], 0)
    upd_item = np.concatenate([results[k]["upd_item"] for k in range(NCORES)], 0)
    return upd_user, upd_item


def kernel(user_embeddings, item_embeddings, adjacency_matrix, W1, W2):
    import time
    import concourse.bass_utils as bass_utils
    nc = _get_nc()
    in_maps = make_in_maps(user_embeddings, item_embeddings, adjacency_matrix,
                           W1, W2)
    last = None
    for attempt in range(3):
        try:
            res = bass_utils.run_bass_kernel_spmd(
                nc, in_maps, core_ids=list(range(NCORES)), trace=False)
            return assemble(res.results)
        except Exception as e:  # transient NRT/axon failures
            last = e
            time.sleep(10)
    raise last



# revision 5
# speedup vs baseline: 1.2794x; 1.2794x over previous
"""Trainium2 Bass kernel for NGCF-style embedding propagation (8 NeuronCores).

Math (reference, with A = adj / (sqrt(row_sum*col_sum)+eps)):
  updated_user = LReLU(A.T @ (item@W1) + (item * (A.T @ user)) @ W2 + user)
  updated_item = LReLU(A   @ (user@W1) + (user * (A   @ item)) @ W2 + item)

Row-shard adj across 8 cores (1024 rows each). Per core, with
Xr = s_r*[iown@W1, uown] (own rows) and Xc = s_c*[user@W1, item] (all cols):
  P^T = Xr^T @ adj  (per 512-col sub)   -> ReduceScatter over user blocks
  Q^T = xc^T @ adjT (accumulated)       -> local (own rows)

v2 vs v1 (both were PE-sequencer bound in the cost model):
 - Q computed in transposed form: one matmul per (panel, j, half) with
   512-wide moving operand spanning 4 row-blocks -> 128 Q matmuls (was 512).
 - ei = [user@W1 | item] built from OWN rows only (uown@W1, iown — already
   loaded) and AllGathered as fp16 with 2KB descriptors, eliminating the
   23us of 256B-descriptor full-embedding loads.
 - item/user finish run in transposed [feat, row] space: the W2 product is
   2 stationary-W2 matmuls over [64, 512] instead of per-block
   transpose+matmul chains; only 8 small output transposes per side.
 - W1/W2/ownT embeddings are mirrored on partitions 64-127 so the upper
   (q1/P1) halves of the transposed accumulators stay partition-aligned.
"""

import numpy as np

N = 8192
D = 64
NCORES = 8
U = N // NCORES          # rows per core = 1024
UB = U // 128            # 128-row blocks per core = 8
CB = N // 128            # 128-col blocks = 64
PAN = 8                  # column panels
PCB = CB // PAN          # col blocks per panel = 8
PW = PCB * 128           # panel width = 1024

_CACHE = {}


def _build(dbg=False, single=False):
    import concourse.bass as bass
    import concourse.bacc as bacc
    import concourse.mybir as mybir
    import concourse.tile as tile
    from concourse import masks

    f32 = mybir.dt.float32
    f16 = mybir.dt.float16
    AF = mybir.ActivationFunctionType
    ALU = mybir.AluOpType
    ds = bass.ds

    nc = bacc.Bacc("TRN2", target_bir_lowering=False, debug=False,
                   num_devices=(1 if single else NCORES), enable_asserts=False)

    adj = nc.dram_tensor("adj", [U, N], f32, kind="ExternalInput").ap()
    user_own = nc.dram_tensor("user_own", [U, D], f32, kind="ExternalInput").ap()
    item_own = nc.dram_tensor("item_own", [U, D], f32, kind="ExternalInput").ap()
    w1 = nc.dram_tensor("w1", [D, D], f32, kind="ExternalInput").ap()
    w2 = nc.dram_tensor("w2", [D, D], f32, kind="ExternalInput").ap()
    upd_user = nc.dram_tensor("upd_user", [U, D], f32, kind="ExternalOutput").ap()
    upd_item = nc.dram_tensor("upd_item", [U, D], f32, kind="ExternalOutput").ap()

    groups = [list(range(NCORES))]

    with tile.TileContext(nc) as tc:
        with (
            tc.tile_pool(name="persist", bufs=1) as persist,
            tc.tile_pool(name="ld", bufs=5) as ldp,
            tc.tile_pool(name="small", bufs=2) as small,
            tc.tile_pool(name="fin", bufs=1) as fin,
            tc.tile_pool(name="pstp", bufs=2) as pstp,
            tc.tile_pool(name="ps_sm", bufs=1, space="PSUM") as ps_sm,
            tc.tile_pool(name="ps2k", bufs=4, space="PSUM") as ps2k,
            tc.tile_pool(name="ps_q", bufs=1, space="PSUM") as ps_q,
            tc.tile_pool(name="dram", bufs=1, space="DRAM") as dram,
        ):
            # ---------------- persistent SBUF tiles
            cache = persist.tile([128, UB, N], f16)          # 128 KiB/part
            adjt = persist.tile([128, 2, PCB, UB, 128], f16)  # 32 KiB
            ei = persist.tile([128, CB, 2 * D], f16)         # 16 KiB (e1|item)
            xc = persist.tile([128, PCB, 2 * D], f16)        # 2 KiB
            uown = persist.tile([128, UB, D], f16)           # 1
            iown = persist.tile([128, UB, D], f16)           # 1
            uown_t = persist.tile([128, UB, 128], f16)       # 2 (parts 64:128)
            iown_t = persist.tile([128, UB, 128], f16)       # 2 (parts 64:128)
            xr = persist.tile([128, UB, 2 * D], f16)         # 2 (x0r then Xr)
            eist = persist.tile([128, UB, 2 * D], f16)       # 2 ([uW1|iown] own)

            s_r = persist.tile([128, UB], f32)
            s_c = persist.tile([128, CB], f32)
            out_stage = persist.tile([128, UB, D], f16)      # 1
            w1_hi = persist.tile([128, D], f16)              # parts 64:128
            w2_hi = persist.tile([128, D], f16)              # parts 64:128
            ones_hf = persist.tile([128, 1], f16)
            onerow = persist.tile([1, 128], f16)
            zrow = persist.tile([1, CB + UB], f16)
            ident = persist.tile([128, 128], f16)
            pt_sb = eist                                     # reuse (dead then)

            psum_qt = ps_q.tile([128, UB, 128], f32)         # 2 banks, Q^T
            psum_cr = ps_q.tile([128, CB + UB], f32)         # col+row sums

            nc.gpsimd.memset(ones_hf[:], 1.0)
            nc.gpsimd.memset(onerow[:], 1.0)
            nc.gpsimd.memset(zrow[:], 0.0)
            masks.make_identity(nc, ident[:])
            # prime psum_cr: one zero-matmul start=True opens a single
            # accumulation group for every col/row-sum region; all later
            # free-size-1 matmuls accumulate with start=False
            nc.tensor.matmul(psum_cr[:], onerow[:], zrow[:],
                             start=True, stop=False, skip_group_check=True)

            # W1/W2 -> fp16 on partitions 64:128 (stationary for the
            # transposed-space matmuls whose moving operand sits there)
            for wsrc, wdst in ((w1, w1_hi), (w2, w2_hi)):
                wld = small.tile([128, D], f32, tag="wld")
                nc.gpsimd.dma_start(wld[64:128], wsrc)
                nc.vector.tensor_copy(wdst[64:128], wld[64:128])

            # own embeddings (fp32 in DRAM, fp16 in SBUF via DMA convert)
            uo_view = user_own.rearrange("(ub p) d -> p ub d", p=128)
            io_view = item_own.rearrange("(ub p) d -> p ub d", p=128)
            nc.gpsimd.dma_start(uown[:], uo_view)
            nc.gpsimd.dma_start(iown[:], io_view)

            # setup compute, emitted at panel-0 hooks so the early adjacency
            # casts aren't queued behind it
            def setup_ownt():
                # transposed own embeddings on partitions 64:128
                for ub in range(UB):
                    pt = ps_sm.tile([128, 2, 128], f16, tag="sm")
                    nc.tensor.transpose(pt[64:128, 0], uown[:, ub], ident[:])
                    nc.tensor.transpose(pt[64:128, 1], iown[:, ub], ident[:])
                    nc.vector.tensor_copy(uown_t[64:128, ub], pt[64:128, 0])
                    nc.vector.tensor_copy(iown_t[64:128, ub], pt[64:128, 1])

            def setup_eist():
                # eist = [uown@W1 | iown]; x0r = [iown@W1 | uown]
                for ub in range(UB):
                    pe = ps_sm.tile([128, 2, D], f32, tag="sm")
                    nc.tensor.matmul(pe[:, 0], uown_t[64:128, ub],
                                     w1_hi[64:128], start=True, stop=True)
                    nc.tensor.matmul(pe[:, 1], iown_t[64:128, ub],
                                     w1_hi[64:128], start=True, stop=True)
                    nc.scalar.activation(eist[:, ub, 0:D], pe[:, 0], AF.Copy)
                    nc.scalar.activation(xr[:, ub, 0:D], pe[:, 1], AF.Copy)
                nc.vector.tensor_copy(
                    eist[:].rearrange("p ub (h d) -> p ub h d", h=2)[:, :, 1],
                    iown[:])
                nc.vector.tensor_copy(
                    xr[:].rearrange("p ub (h d) -> p ub h d", h=2)[:, :, 1],
                    uown[:])

            # AllGather ei = [user@W1 | item] as fp16 (2KB descriptors)
            ei_in = dram.tile([128, UB, 2 * D], f16, name="ei_in")
            ei_ag = dram.tile([NCORES, 128, UB, 2 * D], f16,
                              addr_space="Shared", name="ei_ag")

            def setup_ag():
                nc.gpsimd.dma_start(ei_in[:], eist[:])
                if single:
                    nc.gpsimd.dma_start(ei_ag[0], ei_in[:])
                else:
                    nc.gpsimd.collective_compute(
                        "AllGather", mybir.AluOpType.bypass,
                        replica_groups=groups,
                        ins=[ei_in.opt()], outs=[ei_ag.opt()])
                nc.gpsimd.dma_start(
                    ei[:].rearrange("p (g ub) f -> p g ub f", g=NCORES),
                    ei_ag.rearrange("g p ub f -> p g ub f"))

            # per-panel column-sum AllReduce buffers
            col_in = []
            col_out = []
            for _pn in range(PAN):
                ci = dram.tile([128, PCB], f32, name=f"col_in{_pn}")
                co = dram.tile([128, PCB], f32, addr_space="Shared",
                               name=f"col_out{_pn}")
                col_in.append(ci)
                col_out.append(co)

            adj_v = adj.rearrange("(ub p) n -> p ub n", p=128)

            def emit_q(panel, j0=0, j1=PCB):
                """Q^T matmuls for a completed panel (lagged): accumulate
                [2D, (ub r)] with 512-wide moving operands."""
                buf = panel % 2
                for j in range(j0, j1):
                    st = (panel == 0 and j == 0)
                    sp = False
                    nc.tensor.matmul(psum_qt[:, 0:4], xc[:, j],
                                     adjt[:, buf, j, 0:4],
                                     start=st, stop=sp, skip_group_check=True)
                    nc.tensor.matmul(psum_qt[:, 4:8], xc[:, j],
                                     adjt[:, buf, j, 4:8],
                                     start=st, stop=sp, skip_group_check=True)

            colsb2 = [None] * PAN

            def emit_sc_xc(panel):
                """s_c + Xc for a panel whose AllReduce result is back."""
                csl = slice(panel * PCB, (panel + 1) * PCB)
                sqc = small.tile([128, PCB], f32, tag="sqc")
                nc.scalar.sqrt(sqc[:], colsb2[panel][:])
                nc.vector.reciprocal(s_c[:, csl], sqc[:])
                for j in range(PCB):
                    cb = panel * PCB + j
                    nc.vector.tensor_scalar(
                        xc[:, j], ei[:, cb], s_c[:, cb:cb + 1],
                        None, ALU.mult)

            def emit_rowsums(pan, ub):
                """Row-sum partials from the transposed blocks: free-size-1
                matmuls (engine-free); lagged 2 chunks so the adjT staging
                copy is guaranteed done."""
                buf = pan % 2
                for j in range(PCB):
                    nc.tensor.matmul(
                        psum_cr[:, CB + ub:CB + ub + 1],
                        adjt[:, buf, j, ub], ones_hf[:],
                        start=False,
                        stop=(pan == PAN - 1 and j == PCB - 1),
                        skip_group_check=True)

            # ---------------- phase A: panel-major streaming
            chunk_hist = []
            for panel in range(PAN):
                for ub in range(UB):
                    # lag-2 panel chain: the AllReduce roundtrip takes more
                    # than one panel on the Pool SWDGE queue, so consume its
                    # result (s_c -> xc -> Q) two panels later, just before
                    # this panel's first adjT write (order guards the WAR)
                    if ub == 0 and panel >= 2:
                        emit_sc_xc(panel - 2)
                        emit_q(panel - 2)
                    chunk_hist.append((panel, ub))
                    if len(chunk_hist) > 2:
                        emit_rowsums(*chunk_hist[-3])
                    pst = ps2k.tile([128, PCB, 128], f16, tag="s2k")
                    for half in range(2):
                        hw = PW // 2
                        c0h = panel * PW + half * hw
                        ld = ldp.tile([128, hw], f32, tag="ld")
                        nc.sync.dma_start(ld[:], adj_v[:, ub, c0h:c0h + hw])
                        nc.scalar.activation(
                            cache[:, ub, c0h:c0h + hw], ld[:], AF.Copy)
                        # PE transposes -> PSUM (fp16), staged to adjT on DVE
                        for jh in range(PCB // 2):
                            j = half * (PCB // 2) + jh
                            c0 = panel * PW + j * 128
                            nc.tensor.transpose(pst[:, j],
                                                cache[:, ub, c0:c0 + 128],
                                                ident[:])
                        # column partial sums (free-size-1 matmuls: ~0 engine)
                        for jh in range(PCB // 2):
                            j = half * (PCB // 2) + jh
                            cb = panel * PCB + j
                            c0 = cb * 128
                            nc.tensor.matmul(
                                psum_cr[:, cb:cb + 1],
                                cache[:, ub, c0:c0 + 128], ones_hf[:],
                                start=False,
                                stop=(panel == PAN - 1 and ub == UB - 1),
                                skip_group_check=True)
                    nc.vector.tensor_copy(adjt[:, panel % 2, :, ub], pst[:])
                    # setup compute hooks (panel 0) and the lagged per-panel
                    # chain, emitted late enough that the AllReduce is
                    # already back (no queue-head stall)
                    if panel == 0:
                        if ub == 1:
                            setup_ownt()
                        elif ub == 2:
                            setup_eist()
                        elif ub == 3:
                            setup_ag()

                # panel column sums complete -> AllReduce (latency hidden)
                csl = slice(panel * PCB, (panel + 1) * PCB)
                col_sb = small.tile([128, PCB], f32, tag="colsb")
                nc.vector.tensor_copy(col_sb[:], psum_cr[:, csl])
                nc.gpsimd.dma_start(col_in[panel][:], col_sb[:])
                if single:
                    nc.gpsimd.dma_start(col_out[panel][:], col_in[panel][:])
                else:
                    nc.gpsimd.collective_compute(
                        "AllReduce", mybir.AluOpType.add, replica_groups=groups,
                        ins=[col_in[panel].opt()], outs=[col_out[panel].opt()])
                cb2 = small.tile([128, PCB], f32, tag="cs2", name=f"cs2_{panel}")
                colsb2[panel] = cb2
                nc.gpsimd.dma_start(cb2[:], col_out[panel][:])

            # ---------------- tail
            emit_rowsums(*chunk_hist[-2])
            emit_rowsums(*chunk_hist[-1])

            # s_r and Xr (scale x0r in place)
            sqr = small.tile([128, UB], f32, tag="sqr2")
            nc.scalar.sqrt(sqr[:], psum_cr[:, CB:CB + UB])
            nc.vector.reciprocal(s_r[:], sqr[:])
            for ub in range(UB):
                nc.scalar.activation(xr[:, ub], xr[:, ub], AF.Copy,
                                     scale=s_r[:, ub:ub + 1])

            def emit_item_finish():
                """out_item = LReLU(s_r*(q0 + (q1*uown)@W2) + iown), done in
                transposed space: q0T/q1T = psum_qt[0:64]/[64:128]."""
                for h in range(2):
                    hsl = slice(4 * h, 4 * (h + 1))
                    g = fin.tile([128, 4, 128], f16, tag="g")
                    nc.vector.tensor_mul(g[64:128], psum_qt[64:128, hsl],
                                         uown_t[64:128, hsl])
                    # accumulate (q1*uown)@W2 directly onto q0T in PSUM
                    nc.tensor.matmul(psum_qt[0:64, hsl], w2_hi[64:128],
                                     g[64:128], start=False, stop=True,
                                     skip_group_check=True)
                    sh = fin.tile([64, 4, 128], f16, tag="sh")
                    nc.vector.tensor_copy(sh[:], psum_qt[0:64, hsl])
                    tr_ps = ps_sm.tile([128, 4, D], f16, tag="sm")
                    for k in range(4):
                        ub = 4 * h + k
                        nc.tensor.transpose(tr_ps[:, k], sh[:, k],
                                            ident[0:64, 0:64])
                    for k in range(4):
                        ub = 4 * h + k
                        tb = small.tile([128, D], f32, tag="ft")
                        nc.vector.scalar_tensor_tensor(
                            tb[:], tr_ps[:, k], s_r[:, ub:ub + 1],
                            iown[:, ub], ALU.mult, ALU.add)
                        nc.vector.scalar_tensor_tensor(
                            out_stage[:, ub], tb[:], 0.2, tb[:],
                            ALU.mult, ALU.max)
                ui_view = upd_item.rearrange("(ub p) d -> p ub d", p=128)
                nc.gpsimd.dma_start(ui_view[:], out_stage[:])

            # P^T: stationary Xr[ub], moving natural cache; 512B-desc pairs.
            # Even subs feed p_in_a (each core's pairs 4g,4g+1 = ub 0-3),
            # odd subs feed p_in_b (ub 4-7); evens run first so the first
            # ReduceScatter + readback + user finish overlap the odd half.
            p_in_a = dram.tile([NCORES, 2, 128, 256], f16, name="p_in_a")
            p_in_b = dram.tile([NCORES, 2, 128, 256], f16, name="p_in_b")
            p_out_a = dram.tile([2, 128, 256], f16, name="p_out_a")
            p_out_b = dram.tile([2, 128, 256], f16, name="p_out_b")

            def emit_rs(p_in_t, p_out_t):
                if single:
                    nc.sync.dma_start(p_out_t[:], p_in_t[0])
                else:
                    nc.gpsimd.collective_compute(
                        "ReduceScatter", mybir.AluOpType.add,
                        replica_groups=groups,
                        ins=[p_in_t.opt()], outs=[p_out_t.opt()])

            pid = nc.vector.partition_id()
            uu_view = upd_user.rearrange("(ub p) d -> p ub d", p=128)

            def finish_user(h):
                """out_user = LReLU(s_c*(P0 + (P1*iown)@W2) + uown), half h,
                in transposed space directly from the ReduceScatter output."""
                hsl = slice(4 * h, 4 * (h + 1))
                p_out_t = p_out_a if h == 0 else p_out_b
                nc.sync.dma_start(
                    pt_sb[:, hsl].rearrange("p (b x) c -> p b (x c)", x=2),
                    p_out_t.rearrange("b d c -> d b c"))
                g2 = fin.tile([128, 4, 128], f16, tag="g")
                nc.vector.tensor_mul(g2[64:128],
                                     pt_sb[64:128, hsl],
                                     iown_t[64:128, hsl])
                ph2 = ps2k.tile([64, 4, 128], f32, tag="s2k")
                nc.tensor.matmul(ph2[:], w2_hi[64:128], g2[64:128],
                                 start=True, stop=True)
                sh2 = fin.tile([64, 4, 128], f16, tag="sh")
                nc.vector.scalar_tensor_tensor(
                    sh2[:], pt_sb[0:64, hsl], 1.0, ph2[:], ALU.mult, ALU.add)
                tr2 = ps_sm.tile([128, 4, D], f16, tag="sm")
                for k in range(4):
                    nc.tensor.transpose(tr2[:, k], sh2[:, k],
                                        ident[0:64, 0:64])
                for k in range(4):
                    ub = 4 * h + k
                    if single:
                        sc_ap = s_c[:, ub:ub + 1]
                    else:
                        sc_ap = s_c[:, ds(pid * UB + ub, 1)]
                    t1 = small.tile([128, D], f32, tag="ft")
                    nc.vector.scalar_tensor_tensor(
                        t1[:], tr2[:, k], sc_ap, uown[:, ub],
                        ALU.mult, ALU.add)
                    nc.vector.scalar_tensor_tensor(
                        out_stage[:, ub], t1[:], 0.2, t1[:],
                        ALU.mult, ALU.max)
                nc.gpsimd.dma_start(uu_view[:, hsl], out_stage[:, hsl])

            sub_order = [2 * t for t in range(8)] + [2 * t + 1 for t in range(8)]
            for t, sub in enumerate(sub_order):
                pp = ps2k.tile([128, 512], f32, tag="s2k")
                for ub in range(UB):
                    nc.tensor.matmul(
                        pp[:], xr[:, ub], cache[:, ub, sub * 512:(sub + 1) * 512],
                        start=(ub == 0), stop=(ub == UB - 1),
                        skip_group_check=True)
                pcast = pstp.tile([128, 2, 256], f16, tag="pst")
                if t % 2:
                    nc.vector.tensor_copy(pcast[:], pp[:])
                else:
                    nc.scalar.activation(pcast[:], pp[:], AF.Copy)
                p_in_t = p_in_a if sub % 2 == 0 else p_in_b
                nc.sync.dma_start(
                    p_in_t[sub // 2].rearrange("b d c -> d b c"), pcast[:])
                if t == 0:
                    emit_sc_xc(PAN - 2)
                    emit_q(PAN - 2, 0, 4)
                elif t == 1:
                    emit_q(PAN - 2, 4, 8)
                elif t == 3:
                    emit_sc_xc(PAN - 1)
                elif t == 4:
                    emit_q(PAN - 1, 0, 4)
                elif t == 5:
                    emit_q(PAN - 1, 4, 8)
                elif t == 7:
                    emit_rs(p_in_a, p_out_a)
                elif t == 8:
                    emit_item_finish()
                elif t == 11:
                    finish_user(0)
            emit_rs(p_in_b, p_out_b)
            finish_user(1)

    nc.compile()
    return nc


def _get_nc(dbg=False):
    key = ("nc", dbg)
    if key not in _CACHE:
        _CACHE[key] = _build(dbg)
    return _CACHE[key]


def make_in_maps(user_embeddings, item_embeddings, adjacency_matrix, W1, W2):
    adj = np.ascontiguousarray(np.asarray(adjacency_matrix, dtype=np.float32))
    ue = np.ascontiguousarray(np.asarray(user_embeddings, dtype=np.float32))
    ie = np.ascontiguousarray(np.asarray(item_embeddings, dtype=np.float32))
    w1 = np.ascontiguousarray(np.asarray(W1, dtype=np.float32))
    w2 = np.ascontiguousarray(np.asarray(W2, dtype=np.float32))
    in_maps = []
    for k in range(NCORES):
        sl = slice(k * U, (k + 1) * U)
        in_maps.append({
            "adj": np.ascontiguousarray(adj[sl]),
            "user_own": np.ascontiguousarray(ue[sl]),
            "item_own": np.ascontiguousarray(ie[sl]),
            "w1": w1,
            "w2": w2,
        })
    return in_maps


def assemble(results):
    upd_user = np.concatenate([results[k]["upd_user"] for k in range(NCORES)], 0)
    upd_item = np.concatenate([results[k]["upd_item"] for k in range(NCORES)], 0)
    return upd_user, upd_item


def kernel(user_embeddings, item_embeddings, adjacency_matrix, W1, W2):
    import time
    import concourse.bass_utils as bass_utils
    nc = _get_nc()
    in_maps = make_in_maps(user_embeddings, item_embeddings, adjacency_matrix,
                           W1, W2)
    last = None
    for attempt in range(3):
        try:
            res = bass_utils.run_bass_kernel_spmd(
                nc, in_maps, core_ids=list(range(NCORES)), trace=False)
            return assemble(res.results)
        except Exception as e:  # transient NRT/axon failures
            last = e
            time.sleep(10)
    raise last


# revision 6
# speedup vs baseline: 1.2883x; 1.0070x over previous
"""Trainium2 Bass kernel for NGCF-style embedding propagation (8 NeuronCores).

Math (reference, with A = adj / (sqrt(row_sum*col_sum)+eps)):
  updated_user = LReLU(A.T @ (item@W1) + (item * (A.T @ user)) @ W2 + user)
  updated_item = LReLU(A   @ (user@W1) + (user * (A   @ item)) @ W2 + item)

Row-shard adj across 8 cores (1024 rows each). Per core, with
Xr = s_r*[iown@W1, uown] (own rows) and Xc = s_c*[user@W1, item] (all cols):
  P^T = Xr^T @ adj  (per 512-col sub)   -> ReduceScatter over user blocks
  Q^T = xc^T @ adjT (accumulated)       -> local (own rows)

v2 vs v1 (both were PE-sequencer bound in the cost model):
 - Q computed in transposed form: one matmul per (panel, j, half) with
   512-wide moving operand spanning 4 row-blocks -> 128 Q matmuls (was 512).
 - ei = [user@W1 | item] built from OWN rows only (uown@W1, iown — already
   loaded) and AllGathered as fp16 with 2KB descriptors, eliminating the
   23us of 256B-descriptor full-embedding loads.
 - item/user finish run in transposed [feat, row] space: the W2 product is
   2 stationary-W2 matmuls over [64, 512] instead of per-block
   transpose+matmul chains; only 8 small output transposes per side.
 - W1/W2/ownT embeddings are mirrored on partitions 64-127 so the upper
   (q1/P1) halves of the transposed accumulators stay partition-aligned.
"""

import numpy as np

N = 8192
D = 64
NCORES = 8
U = N // NCORES          # rows per core = 1024
UB = U // 128            # 128-row blocks per core = 8
CB = N // 128            # 128-col blocks = 64
PAN = 8                  # column panels
PCB = CB // PAN          # col blocks per panel = 8
PW = PCB * 128           # panel width = 1024

_CACHE = {}


def _build(dbg=False, single=False):
    import concourse.bass as bass
    import concourse.bacc as bacc
    import concourse.mybir as mybir
    import concourse.tile as tile
    from concourse import masks

    f32 = mybir.dt.float32
    f16 = mybir.dt.float16
    AF = mybir.ActivationFunctionType
    ALU = mybir.AluOpType
    ds = bass.ds

    nc = bacc.Bacc("TRN2", target_bir_lowering=False, debug=False,
                   num_devices=(1 if single else NCORES), enable_asserts=False)

    adj = nc.dram_tensor("adj", [U, N], f32, kind="ExternalInput").ap()
    user_own = nc.dram_tensor("user_own", [U, D], f32, kind="ExternalInput").ap()
    item_own = nc.dram_tensor("item_own", [U, D], f32, kind="ExternalInput").ap()
    w1 = nc.dram_tensor("w1", [D, D], f32, kind="ExternalInput").ap()
    w2 = nc.dram_tensor("w2", [D, D], f32, kind="ExternalInput").ap()
    upd_user = nc.dram_tensor("upd_user", [U, D], f32, kind="ExternalOutput").ap()
    upd_item = nc.dram_tensor("upd_item", [U, D], f32, kind="ExternalOutput").ap()

    groups = [list(range(NCORES))]

    with tile.TileContext(nc) as tc:
        with (
            tc.tile_pool(name="persist", bufs=1) as persist,
            tc.tile_pool(name="ld", bufs=5) as ldp,
            tc.tile_pool(name="small", bufs=2) as small,
            tc.tile_pool(name="fin", bufs=1) as fin,
            tc.tile_pool(name="pstp", bufs=2) as pstp,
            tc.tile_pool(name="ps_sm", bufs=1, space="PSUM") as ps_sm,
            tc.tile_pool(name="ps2k", bufs=4, space="PSUM") as ps2k,
            tc.tile_pool(name="ps_q", bufs=1, space="PSUM") as ps_q,
            tc.tile_pool(name="dram", bufs=1, space="DRAM") as dram,
        ):
            # ---------------- persistent SBUF tiles
            cache = persist.tile([128, UB, N], f16)          # 128 KiB/part
            adjt = persist.tile([128, 2, PCB, UB, 128], f16)  # 32 KiB
            ei = persist.tile([128, CB, 2 * D], f16)         # 16 KiB (e1|item)
            xc = persist.tile([128, PCB, 2 * D], f16)        # 2 KiB
            uown = persist.tile([128, UB, D], f16)           # 1
            iown = persist.tile([128, UB, D], f16)           # 1
            uown_t = persist.tile([128, UB, 128], f16)       # 2 (parts 64:128)
            iown_t = persist.tile([128, UB, 128], f16)       # 2 (parts 64:128)
            xr = persist.tile([128, UB, 2 * D], f16)         # 2 (x0r then Xr)
            eist = persist.tile([128, UB, 2 * D], f16)       # 2 ([uW1|iown] own)

            s_r = persist.tile([128, UB], f32)
            s_c = persist.tile([128, CB], f32)
            out_stage = persist.tile([128, UB, D], f16)      # 1
            w1_hi = persist.tile([128, D], f16)              # parts 64:128
            w2_hi = persist.tile([128, D], f16)              # parts 64:128
            ones_hf = persist.tile([128, 1], f16)
            onerow = persist.tile([1, 128], f16)
            zrow = persist.tile([1, CB + UB], f16)
            ident = persist.tile([128, 128], f16)
            pt_sb = eist                                     # reuse (dead then)

            psum_qt = ps_q.tile([128, UB, 128], f32)         # 2 banks, Q^T
            psum_cr = ps_q.tile([128, CB + UB], f32)         # col+row sums

            nc.gpsimd.memset(ones_hf[:], 1.0)
            nc.gpsimd.memset(onerow[:], 1.0)
            nc.gpsimd.memset(zrow[:], 0.0)
            masks.make_identity(nc, ident[:])
            # prime psum_cr: one zero-matmul start=True opens a single
            # accumulation group for every col/row-sum region; all later
            # free-size-1 matmuls accumulate with start=False
            nc.tensor.matmul(psum_cr[:], onerow[:], zrow[:],
                             start=True, stop=False, skip_group_check=True)

            # W1/W2 -> fp16 on partitions 64:128 (stationary for the
            # transposed-space matmuls whose moving operand sits there)
            for wsrc, wdst in ((w1, w1_hi), (w2, w2_hi)):
                wld = small.tile([128, D], f32, tag="wld")
                nc.gpsimd.dma_start(wld[64:128], wsrc)
                nc.vector.tensor_copy(wdst[64:128], wld[64:128])

            # own embeddings (fp32 in DRAM, fp16 in SBUF via DMA convert)
            uo_view = user_own.rearrange("(ub p) d -> p ub d", p=128)
            io_view = item_own.rearrange("(ub p) d -> p ub d", p=128)
            nc.gpsimd.dma_start(uown[:], uo_view)
            nc.gpsimd.dma_start(iown[:], io_view)

            # setup compute, emitted at panel-0 hooks so the early adjacency
            # casts aren't queued behind it
            def setup_ownt():
                # transposed own embeddings on partitions 64:128
                for ub in range(UB):
                    pt = ps_sm.tile([128, 2, 128], f16, tag="sm")
                    nc.tensor.transpose(pt[64:128, 0], uown[:, ub], ident[:])
                    nc.tensor.transpose(pt[64:128, 1], iown[:, ub], ident[:])
                    nc.vector.tensor_copy(uown_t[64:128, ub], pt[64:128, 0])
                    nc.vector.tensor_copy(iown_t[64:128, ub], pt[64:128, 1])

            def setup_eist(ubs):
                # eist = [uown@W1 | iown]; x0r = [iown@W1 | uown]
                for ub in ubs:
                    pe = ps_sm.tile([128, 2, D], f32, tag="sm")
                    nc.tensor.matmul(pe[:, 0], uown_t[64:128, ub],
                                     w1_hi[64:128], start=True, stop=True)
                    nc.tensor.matmul(pe[:, 1], iown_t[64:128, ub],
                                     w1_hi[64:128], start=True, stop=True)
                    nc.scalar.activation(eist[:, ub, 0:D], pe[:, 0], AF.Copy)
                    nc.scalar.activation(xr[:, ub, 0:D], pe[:, 1], AF.Copy)
                if ubs[-1] == UB - 1:
                    nc.vector.tensor_copy(
                        eist[:].rearrange(
                            "p ub (h d) -> p ub h d", h=2)[:, :, 1],
                        iown[:])
                    nc.vector.tensor_copy(
                        xr[:].rearrange(
                            "p ub (h d) -> p ub h d", h=2)[:, :, 1],
                        uown[:])

            # AllGather ei = [user@W1 | item] as fp16 (2KB descriptors)
            ei_in = dram.tile([128, UB, 2 * D], f16, name="ei_in")
            ei_ag = dram.tile([NCORES, 128, UB, 2 * D], f16,
                              addr_space="Shared", name="ei_ag")

            def setup_ag():
                nc.gpsimd.dma_start(ei_in[:], eist[:])
                if single:
                    nc.gpsimd.dma_start(ei_ag[0], ei_in[:])
                else:
                    nc.gpsimd.collective_compute(
                        "AllGather", mybir.AluOpType.bypass,
                        replica_groups=groups,
                        ins=[ei_in.opt()], outs=[ei_ag.opt()])
                nc.gpsimd.dma_start(
                    ei[:].rearrange("p (g ub) f -> p g ub f", g=NCORES),
                    ei_ag.rearrange("g p ub f -> p g ub f"))

            # per-panel column-sum AllReduce buffers
            col_in = []
            col_out = []
            for _pn in range(PAN):
                ci = dram.tile([128, PCB], f32, name=f"col_in{_pn}")
                co = dram.tile([128, PCB], f32, addr_space="Shared",
                               name=f"col_out{_pn}")
                col_in.append(ci)
                col_out.append(co)

            adj_v = adj.rearrange("(ub p) n -> p ub n", p=128)

            def emit_q(panel, j0=0, j1=PCB):
                """Q^T matmuls for a completed panel (lagged): accumulate
                [2D, (ub r)] with 512-wide moving operands."""
                buf = panel % 2
                for j in range(j0, j1):
                    st = (panel == 0 and j == 0)
                    sp = False
                    nc.tensor.matmul(psum_qt[:, 0:4], xc[:, j],
                                     adjt[:, buf, j, 0:4],
                                     start=st, stop=sp, skip_group_check=True)
                    nc.tensor.matmul(psum_qt[:, 4:8], xc[:, j],
                                     adjt[:, buf, j, 4:8],
                                     start=st, stop=sp, skip_group_check=True)

            colsb2 = [None] * PAN

            def emit_sc_xc(panel):
                """s_c + Xc for a panel whose AllReduce result is back."""
                csl = slice(panel * PCB, (panel + 1) * PCB)
                sqc = small.tile([128, PCB], f32, tag="sqc")
                nc.scalar.sqrt(sqc[:], colsb2[panel][:])
                nc.vector.reciprocal(s_c[:, csl], sqc[:])
                for j in range(PCB):
                    cb = panel * PCB + j
                    nc.vector.tensor_scalar(
                        xc[:, j], ei[:, cb], s_c[:, cb:cb + 1],
                        None, ALU.mult)

            def emit_rowsums(pan, ub):
                """Row-sum partials from the transposed blocks: free-size-1
                matmuls (engine-free); lagged 2 chunks so the adjT staging
                copy is guaranteed done."""
                buf = pan % 2
                for j in range(PCB):
                    nc.tensor.matmul(
                        psum_cr[:, CB + ub:CB + ub + 1],
                        adjt[:, buf, j, ub], ones_hf[:],
                        start=False,
                        stop=(pan == PAN - 1 and j == PCB - 1),
                        skip_group_check=True)

            # ---------------- phase A: panel-major streaming
            chunk_hist = []
            for panel in range(PAN):
                for ub in range(UB):
                    # lag-2 panel chain: the AllReduce roundtrip takes more
                    # than one panel on the Pool SWDGE queue, so consume its
                    # result (s_c -> xc -> Q) two panels later, just before
                    # this panel's first adjT write (order guards the WAR)
                    if ub == 0 and panel >= 2:
                        emit_sc_xc(panel - 2)
                        emit_q(panel - 2)
                    chunk_hist.append((panel, ub))
                    if len(chunk_hist) > 2:
                        emit_rowsums(*chunk_hist[-3])
                    pst = ps2k.tile([128, PCB, 128], f16, tag="s2k")
                    for half in range(2):
                        hw = PW // 2
                        c0h = panel * PW + half * hw
                        ld = ldp.tile([128, hw], f32, tag="ld")
                        nc.sync.dma_start(ld[:], adj_v[:, ub, c0h:c0h + hw])
                        nc.scalar.activation(
                            cache[:, ub, c0h:c0h + hw], ld[:], AF.Copy)
                        # PE transposes -> PSUM (fp16), staged to adjT on DVE
                        for jh in range(PCB // 2):
                            j = half * (PCB // 2) + jh
                            c0 = panel * PW + j * 128
                            nc.tensor.transpose(pst[:, j],
                                                cache[:, ub, c0:c0 + 128],
                                                ident[:])
                        # column partial sums (free-size-1 matmuls: ~0 engine)
                        for jh in range(PCB // 2):
                            j = half * (PCB // 2) + jh
                            cb = panel * PCB + j
                            c0 = cb * 128
                            nc.tensor.matmul(
                                psum_cr[:, cb:cb + 1],
                                cache[:, ub, c0:c0 + 128], ones_hf[:],
                                start=False,
                                stop=(panel == PAN - 1 and ub == UB - 1),
                                skip_group_check=True)
                    nc.vector.tensor_copy(adjt[:, panel % 2, :, ub], pst[:])
                    # setup compute hooks (panel 0) and the lagged per-panel
                    # chain, emitted late enough that the AllReduce is
                    # already back (no queue-head stall)
                    if panel == 0:
                        if ub == 1:
                            setup_ownt()
                        elif ub == 2:
                            setup_eist(list(range(4)))
                        elif ub == 3:
                            setup_eist(list(range(4, UB)))
                        elif ub == 4:
                            setup_ag()

                # panel column sums complete -> AllReduce (latency hidden)
                csl = slice(panel * PCB, (panel + 1) * PCB)
                col_sb = small.tile([128, PCB], f32, tag="colsb")
                nc.vector.tensor_copy(col_sb[:], psum_cr[:, csl])
                nc.scalar.dma_start(col_in[panel][:], col_sb[:])
                if single:
                    nc.gpsimd.dma_start(col_out[panel][:], col_in[panel][:])
                else:
                    nc.gpsimd.collective_compute(
                        "AllReduce", mybir.AluOpType.add, replica_groups=groups,
                        ins=[col_in[panel].opt()], outs=[col_out[panel].opt()])
                cb2 = small.tile([128, PCB], f32, tag="cs2", name=f"cs2_{panel}")
                colsb2[panel] = cb2
                nc.gpsimd.dma_start(cb2[:], col_out[panel][:])

            # ---------------- tail
            emit_rowsums(*chunk_hist[-2])
            emit_rowsums(*chunk_hist[-1])

            # s_r and Xr (scale x0r in place)
            sqr = small.tile([128, UB], f32, tag="sqr2")
            nc.scalar.sqrt(sqr[:], psum_cr[:, CB:CB + UB])
            nc.vector.reciprocal(s_r[:], sqr[:])
            for ub in range(UB):
                nc.scalar.activation(xr[:, ub], xr[:, ub], AF.Copy,
                                     scale=s_r[:, ub:ub + 1])

            def emit_item_finish():
                """out_item = LReLU(s_r*(q0 + (q1*uown)@W2) + iown), done in
                transposed space: q0T/q1T = psum_qt[0:64]/[64:128]."""
                for h in range(2):
                    hsl = slice(4 * h, 4 * (h + 1))
                    g = fin.tile([128, 4, 128], f16, tag="g")
                    nc.vector.tensor_mul(g[64:128], psum_qt[64:128, hsl],
                                         uown_t[64:128, hsl])
                    # accumulate (q1*uown)@W2 directly onto q0T in PSUM
                    nc.tensor.matmul(psum_qt[0:64, hsl], w2_hi[64:128],
                                     g[64:128], start=False, stop=True,
                                     skip_group_check=True)
                    sh = fin.tile([64, 4, 128], f16, tag="sh")
                    nc.vector.tensor_copy(sh[:], psum_qt[0:64, hsl])
                    tr_ps = ps_sm.tile([128, 4, D], f16, tag="sm")
                    for k in range(4):
                        ub = 4 * h + k
                        nc.tensor.transpose(tr_ps[:, k], sh[:, k],
                                            ident[0:64, 0:64])
                    for k in range(4):
                        ub = 4 * h + k
                        tb = small.tile([128, D], f32, tag="ft")
                        nc.vector.scalar_tensor_tensor(
                            tb[:], tr_ps[:, k], s_r[:, ub:ub + 1],
                            iown[:, ub], ALU.mult, ALU.add)
                        nc.vector.scalar_tensor_tensor(
                            out_stage[:, ub], tb[:], 0.2, tb[:],
                            ALU.mult, ALU.max)
                ui_view = upd_item.rearrange("(ub p) d -> p ub d", p=128)
                nc.gpsimd.dma_start(ui_view[:], out_stage[:])

            # P^T: stationary Xr[ub], moving natural cache; 512B-desc pairs.
            # Even subs feed p_in_a (each core's pairs 4g,4g+1 = ub 0-3),
            # odd subs feed p_in_b (ub 4-7); evens run first so the first
            # ReduceScatter + readback + user finish overlap the odd half.
            p_in_a = dram.tile([NCORES, 2, 128, 256], f16, name="p_in_a")
            p_in_b = dram.tile([NCORES, 2, 128, 256], f16, name="p_in_b")
            p_out_a = dram.tile([2, 128, 256], f16, name="p_out_a")
            p_out_b = dram.tile([2, 128, 256], f16, name="p_out_b")

            def emit_rs(p_in_t, p_out_t):
                if single:
                    nc.sync.dma_start(p_out_t[:], p_in_t[0])
                else:
                    nc.gpsimd.collective_compute(
                        "ReduceScatter", mybir.AluOpType.add,
                        replica_groups=groups,
                        ins=[p_in_t.opt()], outs=[p_out_t.opt()])

            pid = nc.vector.partition_id()
            uu_view = upd_user.rearrange("(ub p) d -> p ub d", p=128)

            def finish_user(h):
                """out_user = LReLU(s_c*(P0 + (P1*iown)@W2) + uown), half h,
                in transposed space directly from the ReduceScatter output."""
                hsl = slice(4 * h, 4 * (h + 1))
                p_out_t = p_out_a if h == 0 else p_out_b
                nc.sync.dma_start(
                    pt_sb[:, hsl].rearrange("p (b x) c -> p b (x c)", x=2),
                    p_out_t.rearrange("b d c -> d b c"))
                g2 = fin.tile([128, 4, 128], f16, tag="g")
                nc.vector.tensor_mul(g2[64:128],
                                     pt_sb[64:128, hsl],
                                     iown_t[64:128, hsl])
                ph2 = ps2k.tile([64, 4, 128], f32, tag="s2k")
                nc.tensor.matmul(ph2[:], w2_hi[64:128], g2[64:128],
                                 start=True, stop=True)
                sh2 = fin.tile([64, 4, 128], f16, tag="sh")
                nc.vector.scalar_tensor_tensor(
                    sh2[:], pt_sb[0:64, hsl], 1.0, ph2[:], ALU.mult, ALU.add)
                tr2 = ps_sm.tile([128, 4, D], f16, tag="sm")
                for k in range(4):
                    nc.tensor.transpose(tr2[:, k], sh2[:, k],
                                        ident[0:64, 0:64])
                for k in range(4):
                    ub = 4 * h + k
                    if single:
                        sc_ap = s_c[:, ub:ub + 1]
                    else:
                        sc_ap = s_c[:, ds(pid * UB + ub, 1)]
                    t1 = small.tile([128, D], f32, tag="ft")
                    nc.vector.scalar_tensor_tensor(
                        t1[:], tr2[:, k], sc_ap, uown[:, ub],
                        ALU.mult, ALU.add)
                    nc.vector.scalar_tensor_tensor(
                        out_stage[:, ub], t1[:], 0.2, t1[:],
                        ALU.mult, ALU.max)
                nc.gpsimd.dma_start(uu_view[:, hsl], out_stage[:, hsl])

            sub_order = [2 * t for t in range(8)] + [2 * t + 1 for t in range(8)]
            for t, sub in enumerate(sub_order):
                pp = ps2k.tile([128, 512], f32, tag="s2k")
                for ub in range(UB):
                    nc.tensor.matmul(
                        pp[:], xr[:, ub], cache[:, ub, sub * 512:(sub + 1) * 512],
                        start=(ub == 0), stop=(ub == UB - 1),
                        skip_group_check=True)
                pcast = pstp.tile([128, 2, 256], f16, tag="pst")
                if t % 2:
                    nc.vector.tensor_copy(pcast[:], pp[:])
                else:
                    nc.scalar.activation(pcast[:], pp[:], AF.Copy)
                p_in_t = p_in_a if sub % 2 == 0 else p_in_b
                nc.sync.dma_start(
                    p_in_t[sub // 2].rearrange("b d c -> d b c"), pcast[:])
                if t == 0:
                    emit_sc_xc(PAN - 2)
                    emit_q(PAN - 2, 0, 4)
                elif t == 1:
                    emit_q(PAN - 2, 4, 8)
                elif t == 4:
                    emit_sc_xc(PAN - 1)
                elif t == 5:
                    emit_q(PAN - 1, 0, 4)
                elif t == 6:
                    emit_q(PAN - 1, 4, 8)
                elif t == 7:
                    emit_rs(p_in_a, p_out_a)
                elif t == 8:
                    emit_item_finish()
                elif t == 11:
                    finish_user(0)
            emit_rs(p_in_b, p_out_b)
            finish_user(1)

    nc.compile()
    return nc


def _get_nc(dbg=False):
    key = ("nc", dbg)
    if key not in _CACHE:
        _CACHE[key] = _build(dbg)
    return _CACHE[key]


def make_in_maps(user_embeddings, item_embeddings, adjacency_matrix, W1, W2):
    adj = np.ascontiguousarray(np.asarray(adjacency_matrix, dtype=np.float32))
    ue = np.ascontiguousarray(np.asarray(user_embeddings, dtype=np.float32))
    ie = np.ascontiguousarray(np.asarray(item_embeddings, dtype=np.float32))
    w1 = np.ascontiguousarray(np.asarray(W1, dtype=np.float32))
    w2 = np.ascontiguousarray(np.asarray(W2, dtype=np.float32))
    in_maps = []
    for k in range(NCORES):
        sl = slice(k * U, (k + 1) * U)
        in_maps.append({
            "adj": np.ascontiguousarray(adj[sl]),
            "user_own": np.ascontiguousarray(ue[sl]),
            "item_own": np.ascontiguousarray(ie[sl]),
            "w1": w1,
            "w2": w2,
        })
    return in_maps


def assemble(results):
    upd_user = np.concatenate([results[k]["upd_user"] for k in range(NCORES)], 0)
    upd_item = np.concatenate([results[k]["upd_item"] for k in range(NCORES)], 0)
    return upd_user, upd_item


def kernel(user_embeddings, item_embeddings, adjacency_matrix, W1, W2):
    import time
    import concourse.bass_utils as bass_utils
    nc = _get_nc()
    in_maps = make_in_maps(user_embeddings, item_embeddings, adjacency_matrix,
                           W1, W2)
    last = None
    for attempt in range(3):
        try:
            res = bass_utils.run_bass_kernel_spmd(
                nc, in_maps, core_ids=list(range(NCORES)), trace=False)
            return assemble(res.results)
        except Exception as e:  # transient NRT/axon failures
            last = e
            time.sleep(10)
    raise last


# revision 7
# speedup vs baseline: 1.2911x; 1.0021x over previous
"""Trainium2 Bass kernel for NGCF-style embedding propagation (8 NeuronCores).

Math (reference, with A = adj / (sqrt(row_sum*col_sum)+eps)):
  updated_user = LReLU(A.T @ (item@W1) + (item * (A.T @ user)) @ W2 + user)
  updated_item = LReLU(A   @ (user@W1) + (user * (A   @ item)) @ W2 + item)

Row-shard adj across 8 cores (1024 rows each). Per core, with
Xr = s_r*[iown@W1, uown] (own rows) and Xc = s_c*[user@W1, item] (all cols):
  P^T = Xr^T @ adj  (per 512-col sub)   -> ReduceScatter over user blocks
  Q^T = xc^T @ adjT (accumulated)       -> local (own rows)

v2 vs v1 (both were PE-sequencer bound in the cost model):
 - Q computed in transposed form: one matmul per (panel, j, half) with
   512-wide moving operand spanning 4 row-blocks -> 128 Q matmuls (was 512).
 - ei = [user@W1 | item] built from OWN rows only (uown@W1, iown — already
   loaded) and AllGathered as fp16 with 2KB descriptors, eliminating the
   23us of 256B-descriptor full-embedding loads.
 - item/user finish run in transposed [feat, row] space: the W2 product is
   2 stationary-W2 matmuls over [64, 512] instead of per-block
   transpose+matmul chains; only 8 small output transposes per side.
 - W1/W2/ownT embeddings are mirrored on partitions 64-127 so the upper
   (q1/P1) halves of the transposed accumulators stay partition-aligned.
"""

import numpy as np

N = 8192
D = 64
NCORES = 8
U = N // NCORES          # rows per core = 1024
UB = U // 128            # 128-row blocks per core = 8
CB = N // 128            # 128-col blocks = 64
PAN = 8                  # column panels
PCB = CB // PAN          # col blocks per panel = 8
PW = PCB * 128           # panel width = 1024

_CACHE = {}


def _build(dbg=False, single=False):
    import concourse.bass as bass
    import concourse.bacc as bacc
    import concourse.mybir as mybir
    import concourse.tile as tile
    from concourse import masks

    f32 = mybir.dt.float32
    f16 = mybir.dt.float16
    AF = mybir.ActivationFunctionType
    ALU = mybir.AluOpType
    ds = bass.ds

    nc = bacc.Bacc("TRN2", target_bir_lowering=False, debug=False,
                   num_devices=(1 if single else NCORES), enable_asserts=False)

    adj = nc.dram_tensor("adj", [U, N], f32, kind="ExternalInput").ap()
    user_own = nc.dram_tensor("user_own", [U, D], f32, kind="ExternalInput").ap()
    item_own = nc.dram_tensor("item_own", [U, D], f32, kind="ExternalInput").ap()
    w1 = nc.dram_tensor("w1", [D, D], f32, kind="ExternalInput").ap()
    w2 = nc.dram_tensor("w2", [D, D], f32, kind="ExternalInput").ap()
    upd_user = nc.dram_tensor("upd_user", [U, D], f32, kind="ExternalOutput").ap()
    upd_item = nc.dram_tensor("upd_item", [U, D], f32, kind="ExternalOutput").ap()

    groups = [list(range(NCORES))]

    with tile.TileContext(nc) as tc:
        with (
            tc.tile_pool(name="persist", bufs=1) as persist,
            tc.tile_pool(name="ld", bufs=5) as ldp,
            tc.tile_pool(name="small", bufs=2) as small,
            tc.tile_pool(name="fin", bufs=1) as fin,
            tc.tile_pool(name="pstp", bufs=3) as pstp,
            tc.tile_pool(name="ps_sm", bufs=1, space="PSUM") as ps_sm,
            tc.tile_pool(name="ps2k", bufs=4, space="PSUM") as ps2k,
            tc.tile_pool(name="ps_q", bufs=1, space="PSUM") as ps_q,
            tc.tile_pool(name="dram", bufs=1, space="DRAM") as dram,
        ):
            # ---------------- persistent SBUF tiles
            cache = persist.tile([128, UB, N], f16)          # 128 KiB/part
            adjt = persist.tile([128, 2, PCB, UB, 128], f16)  # 32 KiB
            ei = persist.tile([128, CB, 2 * D], f16)         # 16 KiB (e1|item)
            xc = persist.tile([128, PCB, 2 * D], f16)        # 2 KiB
            uown = persist.tile([128, UB, D], f16)           # 1
            iown = persist.tile([128, UB, D], f16)           # 1
            uown_t = persist.tile([128, UB, 128], f16)       # 2 (parts 64:128)
            iown_t = persist.tile([128, UB, 128], f16)       # 2 (parts 64:128)
            xr = persist.tile([128, UB, 2 * D], f16)         # 2 (x0r then Xr)
            eist = persist.tile([128, UB, 2 * D], f16)       # 2 ([uW1|iown] own)

            s_r = persist.tile([128, UB], f32)
            s_c = persist.tile([128, CB], f32)
            out_stage = persist.tile([128, UB, D], f16)      # 1
            w1_hi = persist.tile([128, D], f16)              # parts 64:128
            w2_hi = persist.tile([128, D], f16)              # parts 64:128
            ones_hf = persist.tile([128, 1], f16)
            onerow = persist.tile([1, 128], f16)
            zrow = persist.tile([1, CB + UB], f16)
            ident = persist.tile([128, 128], f16)
            pt_sb = eist                                     # reuse (dead then)

            psum_qt = ps_q.tile([128, UB, 128], f32)         # 2 banks, Q^T
            psum_cr = ps_q.tile([128, CB + UB], f32)         # col+row sums

            nc.gpsimd.memset(ones_hf[:], 1.0)
            nc.gpsimd.memset(onerow[:], 1.0)
            nc.gpsimd.memset(zrow[:], 0.0)
            masks.make_identity(nc, ident[:])
            # prime psum_cr: one zero-matmul start=True opens a single
            # accumulation group for every col/row-sum region; all later
            # free-size-1 matmuls accumulate with start=False
            nc.tensor.matmul(psum_cr[:], onerow[:], zrow[:],
                             start=True, stop=False, skip_group_check=True)

            # W1/W2 -> fp16 on partitions 64:128 (stationary for the
            # transposed-space matmuls whose moving operand sits there)
            for wsrc, wdst in ((w1, w1_hi), (w2, w2_hi)):
                wld = small.tile([128, D], f32, tag="wld")
                nc.gpsimd.dma_start(wld[64:128], wsrc)
                nc.vector.tensor_copy(wdst[64:128], wld[64:128])

            # own embeddings (fp32 in DRAM, fp16 in SBUF via DMA convert)
            uo_view = user_own.rearrange("(ub p) d -> p ub d", p=128)
            io_view = item_own.rearrange("(ub p) d -> p ub d", p=128)
            nc.gpsimd.dma_start(uown[:], uo_view)
            nc.gpsimd.dma_start(iown[:], io_view)

            # setup compute, emitted at panel-0 hooks so the early adjacency
            # casts aren't queued behind it
            def setup_ownt():
                # transposed own embeddings on partitions 64:128
                for ub in range(UB):
                    pt = ps_sm.tile([128, 2, 128], f16, tag="sm")
                    nc.tensor.transpose(pt[64:128, 0], uown[:, ub], ident[:])
                    nc.tensor.transpose(pt[64:128, 1], iown[:, ub], ident[:])
                    nc.vector.tensor_copy(uown_t[64:128, ub], pt[64:128, 0])
                    nc.vector.tensor_copy(iown_t[64:128, ub], pt[64:128, 1])

            def setup_eist(ubs):
                # eist = [uown@W1 | iown]; x0r = [iown@W1 | uown]
                for ub in ubs:
                    pe = ps_sm.tile([128, 2, D], f32, tag="sm")
                    nc.tensor.matmul(pe[:, 0], uown_t[64:128, ub],
                                     w1_hi[64:128], start=True, stop=True)
                    nc.tensor.matmul(pe[:, 1], iown_t[64:128, ub],
                                     w1_hi[64:128], start=True, stop=True)
                    nc.scalar.activation(eist[:, ub, 0:D], pe[:, 0], AF.Copy)
                    nc.scalar.activation(xr[:, ub, 0:D], pe[:, 1], AF.Copy)
                if ubs[-1] == UB - 1:
                    nc.vector.tensor_copy(
                        eist[:].rearrange(
                            "p ub (h d) -> p ub h d", h=2)[:, :, 1],
                        iown[:])
                    nc.vector.tensor_copy(
                        xr[:].rearrange(
                            "p ub (h d) -> p ub h d", h=2)[:, :, 1],
                        uown[:])

            # AllGather ei = [user@W1 | item] as fp16 (2KB descriptors)
            ei_in = dram.tile([128, UB, 2 * D], f16, name="ei_in")
            ei_ag = dram.tile([NCORES, 128, UB, 2 * D], f16,
                              addr_space="Shared", name="ei_ag")

            def setup_ag():
                nc.gpsimd.dma_start(ei_in[:], eist[:])
                if single:
                    nc.gpsimd.dma_start(ei_ag[0], ei_in[:])
                else:
                    nc.gpsimd.collective_compute(
                        "AllGather", mybir.AluOpType.bypass,
                        replica_groups=groups,
                        ins=[ei_in.opt()], outs=[ei_ag.opt()])
                nc.gpsimd.dma_start(
                    ei[:].rearrange("p (g ub) f -> p g ub f", g=NCORES),
                    ei_ag.rearrange("g p ub f -> p g ub f"))

            # per-panel column-sum AllReduce buffers
            col_in = []
            col_out = []
            for _pn in range(PAN):
                ci = dram.tile([128, PCB], f32, name=f"col_in{_pn}")
                co = dram.tile([128, PCB], f32, addr_space="Shared",
                               name=f"col_out{_pn}")
                col_in.append(ci)
                col_out.append(co)

            adj_v = adj.rearrange("(ub p) n -> p ub n", p=128)

            def emit_q(panel, j0=0, j1=PCB):
                """Q^T matmuls for a completed panel (lagged): accumulate
                [2D, (ub r)] with 512-wide moving operands."""
                buf = panel % 2
                for j in range(j0, j1):
                    st = (panel == 0 and j == 0)
                    sp = False
                    nc.tensor.matmul(psum_qt[:, 0:4], xc[:, j],
                                     adjt[:, buf, j, 0:4],
                                     start=st, stop=sp, skip_group_check=True)
                    nc.tensor.matmul(psum_qt[:, 4:8], xc[:, j],
                                     adjt[:, buf, j, 4:8],
                                     start=st, stop=sp, skip_group_check=True)

            colsb2 = [None] * PAN

            def emit_sc_xc(panel):
                """s_c + Xc for a panel whose AllReduce result is back."""
                csl = slice(panel * PCB, (panel + 1) * PCB)
                sqc = small.tile([128, PCB], f32, tag="sqc")
                nc.scalar.sqrt(sqc[:], colsb2[panel][:])
                nc.vector.reciprocal(s_c[:, csl], sqc[:])
                for j in range(PCB):
                    cb = panel * PCB + j
                    nc.vector.tensor_scalar(
                        xc[:, j], ei[:, cb], s_c[:, cb:cb + 1],
                        None, ALU.mult)

            def emit_rowsums(pan, ub):
                """Row-sum partials from the transposed blocks: free-size-1
                matmuls (engine-free); lagged 2 chunks so the adjT staging
                copy is guaranteed done."""
                buf = pan % 2
                for j in range(PCB):
                    nc.tensor.matmul(
                        psum_cr[:, CB + ub:CB + ub + 1],
                        adjt[:, buf, j, ub], ones_hf[:],
                        start=False,
                        stop=(pan == PAN - 1 and j == PCB - 1),
                        skip_group_check=True)

            # ---------------- phase A: panel-major streaming
            chunk_hist = []
            for panel in range(PAN):
                for ub in range(UB):
                    # lag-2 panel chain: the AllReduce roundtrip takes more
                    # than one panel on the Pool SWDGE queue, so consume its
                    # result (s_c -> xc -> Q) two panels later, just before
                    # this panel's first adjT write (order guards the WAR)
                    if ub == 0 and panel >= 2:
                        emit_sc_xc(panel - 2)
                        emit_q(panel - 2)
                    chunk_hist.append((panel, ub))
                    if len(chunk_hist) > 2:
                        emit_rowsums(*chunk_hist[-3])
                    pst = ps2k.tile([128, PCB, 128], f16, tag="s2k")
                    for half in range(2):
                        hw = PW // 2
                        c0h = panel * PW + half * hw
                        ld = ldp.tile([128, hw], f32, tag="ld")
                        nc.sync.dma_start(ld[:], adj_v[:, ub, c0h:c0h + hw])
                        nc.scalar.activation(
                            cache[:, ub, c0h:c0h + hw], ld[:], AF.Copy)
                        # PE transposes -> PSUM (fp16), staged to adjT on DVE
                        for jh in range(PCB // 2):
                            j = half * (PCB // 2) + jh
                            c0 = panel * PW + j * 128
                            nc.tensor.transpose(pst[:, j],
                                                cache[:, ub, c0:c0 + 128],
                                                ident[:])
                        # column partial sums (free-size-1 matmuls: ~0 engine)
                        for jh in range(PCB // 2):
                            j = half * (PCB // 2) + jh
                            cb = panel * PCB + j
                            c0 = cb * 128
                            nc.tensor.matmul(
                                psum_cr[:, cb:cb + 1],
                                cache[:, ub, c0:c0 + 128], ones_hf[:],
                                start=False,
                                stop=(panel == PAN - 1 and ub == UB - 1),
                                skip_group_check=True)
                    nc.vector.tensor_copy(adjt[:, panel % 2, :, ub], pst[:])
                    # setup compute hooks (panel 0) and the lagged per-panel
                    # chain, emitted late enough that the AllReduce is
                    # already back (no queue-head stall)
                    if panel == 0:
                        if ub == 1:
                            setup_ownt()
                        elif ub == 2:
                            setup_eist(list(range(4)))
                        elif ub == 3:
                            setup_eist(list(range(4, UB)))
                        elif ub == 4:
                            setup_ag()

                # panel column sums complete -> AllReduce (latency hidden)
                csl = slice(panel * PCB, (panel + 1) * PCB)
                col_sb = small.tile([128, PCB], f32, tag="colsb")
                nc.vector.tensor_copy(col_sb[:], psum_cr[:, csl])
                nc.scalar.dma_start(col_in[panel][:], col_sb[:])
                if single:
                    nc.gpsimd.dma_start(col_out[panel][:], col_in[panel][:])
                else:
                    nc.gpsimd.collective_compute(
                        "AllReduce", mybir.AluOpType.add, replica_groups=groups,
                        ins=[col_in[panel].opt()], outs=[col_out[panel].opt()])
                cb2 = small.tile([128, PCB], f32, tag="cs2", name=f"cs2_{panel}")
                colsb2[panel] = cb2
                nc.gpsimd.dma_start(cb2[:], col_out[panel][:])

            # ---------------- tail
            emit_rowsums(*chunk_hist[-2])
            emit_rowsums(*chunk_hist[-1])

            # s_r and Xr (scale x0r in place)
            sqr = small.tile([128, UB], f32, tag="sqr2")
            nc.scalar.sqrt(sqr[:], psum_cr[:, CB:CB + UB])
            nc.vector.reciprocal(s_r[:], sqr[:])
            for ub in range(UB):
                nc.scalar.activation(xr[:, ub], xr[:, ub], AF.Copy,
                                     scale=s_r[:, ub:ub + 1])

            def emit_item_finish():
                """out_item = LReLU(s_r*(q0 + (q1*uown)@W2) + iown), done in
                transposed space: q0T/q1T = psum_qt[0:64]/[64:128]."""
                for h in range(2):
                    hsl = slice(4 * h, 4 * (h + 1))
                    g = fin.tile([128, 4, 128], f16, tag="g")
                    nc.vector.tensor_mul(g[64:128], psum_qt[64:128, hsl],
                                         uown_t[64:128, hsl])
                    # accumulate (q1*uown)@W2 directly onto q0T in PSUM
                    nc.tensor.matmul(psum_qt[0:64, hsl], w2_hi[64:128],
                                     g[64:128], start=False, stop=True,
                                     skip_group_check=True)
                    sh = fin.tile([64, 4, 128], f16, tag="g")
                    nc.vector.tensor_copy(sh[:], psum_qt[0:64, hsl])
                    tr_ps = ps_sm.tile([128, 4, D], f16, tag="sm")
                    for k in range(4):
                        ub = 4 * h + k
                        nc.tensor.transpose(tr_ps[:, k], sh[:, k],
                                            ident[0:64, 0:64])
                    for k in range(4):
                        ub = 4 * h + k
                        tb = small.tile([128, D], f32, tag="ft")
                        nc.vector.scalar_tensor_tensor(
                            tb[:], tr_ps[:, k], s_r[:, ub:ub + 1],
                            iown[:, ub], ALU.mult, ALU.add)
                        nc.vector.scalar_tensor_tensor(
                            out_stage[:, ub], tb[:], 0.2, tb[:],
                            ALU.mult, ALU.max)
                ui_view = upd_item.rearrange("(ub p) d -> p ub d", p=128)
                nc.gpsimd.dma_start(ui_view[:], out_stage[:])

            # P^T: stationary Xr[ub], moving natural cache; 512B-desc pairs.
            # Even subs feed p_in_a (each core's pairs 4g,4g+1 = ub 0-3),
            # odd subs feed p_in_b (ub 4-7); evens run first so the first
            # ReduceScatter + readback + user finish overlap the odd half.
            p_in_a = dram.tile([NCORES, 2, 128, 256], f16, name="p_in_a")
            p_in_b = dram.tile([NCORES, 2, 128, 256], f16, name="p_in_b")
            p_out_a = dram.tile([2, 128, 256], f16, name="p_out_a")
            p_out_b = dram.tile([2, 128, 256], f16, name="p_out_b")

            def emit_rs(p_in_t, p_out_t):
                if single:
                    nc.sync.dma_start(p_out_t[:], p_in_t[0])
                else:
                    nc.gpsimd.collective_compute(
                        "ReduceScatter", mybir.AluOpType.add,
                        replica_groups=groups,
                        ins=[p_in_t.opt()], outs=[p_out_t.opt()])

            pid = nc.vector.partition_id()
            uu_view = upd_user.rearrange("(ub p) d -> p ub d", p=128)

            def finish_user(h):
                """out_user = LReLU(s_c*(P0 + (P1*iown)@W2) + uown), half h,
                in transposed space directly from the ReduceScatter output."""
                hsl = slice(4 * h, 4 * (h + 1))
                p_out_t = p_out_a if h == 0 else p_out_b
                nc.sync.dma_start(
                    pt_sb[:, hsl].rearrange("p (b x) c -> p b (x c)", x=2),
                    p_out_t.rearrange("b d c -> d b c"))
                g2 = fin.tile([128, 4, 128], f16, tag="g")
                nc.vector.tensor_mul(g2[64:128],
                                     pt_sb[64:128, hsl],
                                     iown_t[64:128, hsl])
                ph2 = ps2k.tile([64, 4, 128], f32, tag="s2k")
                nc.tensor.matmul(ph2[:], w2_hi[64:128], g2[64:128],
                                 start=True, stop=True)
                sh2 = fin.tile([64, 4, 128], f16, tag="g")
                nc.vector.scalar_tensor_tensor(
                    sh2[:], pt_sb[0:64, hsl], 1.0, ph2[:], ALU.mult, ALU.add)
                tr2 = ps_sm.tile([128, 4, D], f16, tag="sm")
                for k in range(4):
                    nc.tensor.transpose(tr2[:, k], sh2[:, k],
                                        ident[0:64, 0:64])
                for k in range(4):
                    ub = 4 * h + k
                    if single:
                        sc_ap = s_c[:, ub:ub + 1]
                    else:
                        sc_ap = s_c[:, ds(pid * UB + ub, 1)]
                    t1 = small.tile([128, D], f32, tag="ft")
                    nc.vector.scalar_tensor_tensor(
                        t1[:], tr2[:, k], sc_ap, uown[:, ub],
                        ALU.mult, ALU.add)
                    nc.vector.scalar_tensor_tensor(
                        out_stage[:, ub], t1[:], 0.2, t1[:],
                        ALU.mult, ALU.max)
                nc.gpsimd.dma_start(uu_view[:, hsl], out_stage[:, hsl])

            sub_order = [2 * t for t in range(8)] + [2 * t + 1 for t in range(8)]
            for t, sub in enumerate(sub_order):
                pp = ps2k.tile([128, 512], f32, tag="s2k")
                for ub in range(UB):
                    nc.tensor.matmul(
                        pp[:], xr[:, ub], cache[:, ub, sub * 512:(sub + 1) * 512],
                        start=(ub == 0), stop=(ub == UB - 1),
                        skip_group_check=True)
                pcast = pstp.tile([128, 2, 256], f16, tag="pst")
                if t % 2:
                    nc.vector.tensor_copy(pcast[:], pp[:])
                else:
                    nc.scalar.activation(pcast[:], pp[:], AF.Copy)
                p_in_t = p_in_a if sub % 2 == 0 else p_in_b
                nc.sync.dma_start(
                    p_in_t[sub // 2].rearrange("b d c -> d b c"), pcast[:])
                if t == 0:
                    emit_sc_xc(PAN - 2)
                    emit_q(PAN - 2, 0, 4)
                elif t == 1:
                    emit_q(PAN - 2, 4, 8)
                elif t == 4:
                    emit_sc_xc(PAN - 1)
                elif t == 5:
                    emit_q(PAN - 1, 0, 4)
                elif t == 6:
                    emit_q(PAN - 1, 4, 8)
                elif t == 7:
                    emit_rs(p_in_a, p_out_a)
                elif t == 8:
                    emit_item_finish()
                elif t == 11:
                    finish_user(0)
            emit_rs(p_in_b, p_out_b)
            finish_user(1)

    nc.compile()
    return nc


def _get_nc(dbg=False):
    key = ("nc", dbg)
    if key not in _CACHE:
        _CACHE[key] = _build(dbg)
    return _CACHE[key]


def make_in_maps(user_embeddings, item_embeddings, adjacency_matrix, W1, W2):
    adj = np.ascontiguousarray(np.asarray(adjacency_matrix, dtype=np.float32))
    ue = np.ascontiguousarray(np.asarray(user_embeddings, dtype=np.float32))
    ie = np.ascontiguousarray(np.asarray(item_embeddings, dtype=np.float32))
    w1 = np.ascontiguousarray(np.asarray(W1, dtype=np.float32))
    w2 = np.ascontiguousarray(np.asarray(W2, dtype=np.float32))
    in_maps = []
    for k in range(NCORES):
        sl = slice(k * U, (k + 1) * U)
        in_maps.append({
            "adj": np.ascontiguousarray(adj[sl]),
            "user_own": np.ascontiguousarray(ue[sl]),
            "item_own": np.ascontiguousarray(ie[sl]),
            "w1": w1,
            "w2": w2,
        })
    return in_maps


def assemble(results):
    upd_user = np.concatenate([results[k]["upd_user"] for k in range(NCORES)], 0)
    upd_item = np.concatenate([results[k]["upd_item"] for k in range(NCORES)], 0)
    return upd_user, upd_item


def kernel(user_embeddings, item_embeddings, adjacency_matrix, W1, W2):
    import time
    import concourse.bass_utils as bass_utils
    nc = _get_nc()
    in_maps = make_in_maps(user_embeddings, item_embeddings, adjacency_matrix,
                           W1, W2)
    last = None
    for attempt in range(3):
        try:
            res = bass_utils.run_bass_kernel_spmd(
                nc, in_maps, core_ids=list(range(NCORES)), trace=False)
            return assemble(res.results)
        except Exception as e:  # transient NRT/axon failures
            last = e
            time.sleep(10)
    raise last
